# revision 10
# baseline (speedup 1.0000x reference)
"""EdgeDecoder Trainium2 kernel: out = 5*sigmoid(w2 . relu([z_u[row]; z_m[col]] @ W1.T + b1) + b2).

Strategy (8 NeuronCores, data-parallel over edges):
  1. Algebraic split: z @ W1.T = z_u[row] @ W1u.T + z_m[col] @ W1m.T, so precompute
     node tables A = |w2| * (z_u @ W1u.T + b1), B = |w2| * (z_m @ W1m.T) once on-device
     (PE matmuls), with the hidden dim permuted so positive-w2 units are contiguous.
  2. Per edge: two indirect-DMA row gathers (second accumulates via CCE add),
     relu as max(.,0) on positive block / min(.,0) on negative block (since
     w2[h]*relu(x) = relu(|w2|x) for w2>0 and = min(|w2|x, 0) for w2<0),
     segmented reduce over hidden, pos+neg, sigmoid(+b2) * 5.
Tables are replicated per core; each core processes a contiguous shard of edges.
"""
import sys
import numpy as np

sys.path.insert(0, '/opt/trn_rl_repo')

import concourse.bass as bass
import concourse.bacc as bacc
import concourse.mybir as mybir
import concourse.tile as tile
from concourse.bass_utils import run_bass_kernel_spmd

N_CORES = 8
P = 128
H = 128          # hidden
G = 16           # gather-loop cols per iteration (G*128 edges)
ZBODY = 1024     # precompute rows per loop body

_LAST_STATS = {}


def _build_nc(C, NA, NB, Hp, repeat=1):
    """C: edge cols per core (edges = 128*C). NA/NB: padded table rows. Hp: # pos-w2 units.
    repeat>1 re-runs the compute phases (identical results) for slope-based timing."""
    f32 = mybir.dt.float32
    i32 = mybir.dt.int32
    nc = bacc.Bacc(None, target_bir_lowering=False)

    zTu = nc.dram_tensor("zTu", [P, NA], f32, kind="ExternalInput")
    zTm = nc.dram_tensor("zTm", [P, NB], f32, kind="ExternalInput")
    w1ut = nc.dram_tensor("w1ut", [P, H], f32, kind="ExternalInput")
    w1mt = nc.dram_tensor("w1mt", [P, H], f32, kind="ExternalInput")
    b1rep = nc.dram_tensor("b1rep", [P, H], f32, kind="ExternalInput")
    b2rep = nc.dram_tensor("b2rep", [P, 1], f32, kind="ExternalInput")
    idxA = nc.dram_tensor("idxA", [P, C], i32, kind="ExternalInput")
    idxB = nc.dram_tensor("idxB", [P, C], i32, kind="ExternalInput")
    out_d = nc.dram_tensor("out", [P, C], f32, kind="ExternalOutput")

    tabA = nc.dram_tensor("tabA", [NA, H], f32)
    tabB = nc.dram_tensor("tabB", [NB, H], f32)
    # tile-linearized write view: table row (p*(N/128) + m) <-> partition p, col block m
    tabA_v = tabA[:].rearrange("(p m) d -> p (m d)", p=P)
    tabB_v = tabB[:].rearrange("(p m) d -> p (m d)", p=P)

    with tile.TileContext(nc) as tc:
        with (
            tc.tile_pool(name="const", bufs=1) as cpool,
            tc.tile_pool(name="work", bufs=3) as wpool,
            tc.tile_pool(name="psum", bufs=4, space="PSUM") as ppool,
        ):
            w1ut_t = cpool.tile([P, H], f32)
            w1mt_t = cpool.tile([P, H], f32)
            b1rep_t = cpool.tile([P, H], f32)
            b2rep_t = cpool.tile([P, 1], f32)
            idxA_t = cpool.tile([P, C], i32)
            idxB_t = cpool.tile([P, C], i32)
            logits = cpool.tile([P, C], f32)
            nc.sync.dma_start(out=w1ut_t[:], in_=w1ut[:])
            nc.sync.dma_start(out=w1mt_t[:], in_=w1mt[:])
            nc.sync.dma_start(out=b1rep_t[:], in_=b1rep[:])
            nc.sync.dma_start(out=b2rep_t[:], in_=b2rep[:])
            nc.sync.dma_start(out=idxA_t[:], in_=idxA[:])
            nc.sync.dma_start(out=idxB_t[:], in_=idxB[:])

            # ---- precompute tables ----
            for (zT, w1t, tab_v, npad, addb1) in (
                (zTu, w1ut_t, tabA_v, NA, True),
                (zTm, w1mt_t, tabB_v, NB, False),
            ) * repeat:
                with tc.For_i(0, npad, ZBODY) as iv:
                    zstage = wpool.tile([P, ZBODY], f32, tag="zstage")
                    nc.sync.dma_start(out=zstage[:], in_=zT[:, bass.ds(iv, ZBODY)])
                    astage = wpool.tile([P, ZBODY], f32, tag="astage")
                    for k in range(ZBODY // P):
                        ps = ppool.tile([P, H], f32, tag="ps")
                        nc.tensor.matmul(
                            out=ps[:],
                            lhsT=zstage[:, k * P:(k + 1) * P],
                            rhs=w1t[:],
                            start=True, stop=True,
                        )
                        sl = astage[:, k * H:(k + 1) * H]
                        if addb1:
                            nc.vector.tensor_add(out=sl, in0=ps[:], in1=b1rep_t[:])
                        else:
                            nc.scalar.copy(out=sl, in_=ps[:])
                    nc.sync.dma_start(out=tab_v[:, bass.ds(iv, ZBODY)], in_=astage[:])

            # ---- edge gather + MLP ----
            def gather_body(iv):
                rstage = wpool.tile([P, G], i32, tag="rstage")
                cstage = wpool.tile([P, G], i32, tag="cstage")
                nc.vector.tensor_copy(out=rstage[:], in_=idxA_t[:, bass.ds(iv, G)])
                nc.vector.tensor_copy(out=cstage[:], in_=idxB_t[:, bass.ds(iv, G)])
                ct = wpool.tile([P, G * H], f32, tag="ct")
                for j in range(G):
                    sl = ct[:, j * H:(j + 1) * H]
                    nc.gpsimd.indirect_dma_start(
                        out=sl, out_offset=None, in_=tabA[:],
                        in_offset=bass.IndirectOffsetOnAxis(ap=rstage[:, j:j + 1], axis=0),
                    )
                    nc.gpsimd.indirect_dma_start(
                        out=sl, out_offset=None, in_=tabB[:],
                        in_offset=bass.IndirectOffsetOnAxis(ap=cstage[:, j:j + 1], axis=0),
                        compute_op=mybir.AluOpType.add,
                    )
                cc = ct[:].rearrange("p (g h) -> p g h", h=H)
                if Hp > 0:
                    nc.vector.tensor_scalar_max(out=cc[:, :, 0:Hp], in0=cc[:, :, 0:Hp], scalar1=0.0)
                if Hp < H:
                    nc.vector.tensor_scalar_min(out=cc[:, :, Hp:H], in0=cc[:, :, Hp:H], scalar1=0.0)
                lsl = logits[:, bass.ds(iv, G)]
                if Hp == H or Hp == 0:
                    nc.vector.tensor_reduce(out=lsl, in_=cc[:, :, :], axis=mybir.AxisListType.X,
                                            op=mybir.AluOpType.add)
                else:
                    pos = wpool.tile([P, G], f32, tag="pos")
                    nc.vector.tensor_reduce(out=pos[:], in_=cc[:, :, 0:Hp],
                                            axis=mybir.AxisListType.X, op=mybir.AluOpType.add)
                    neg = wpool.tile([P, G], f32, tag="neg")
                    nc.vector.tensor_reduce(out=neg[:], in_=cc[:, :, Hp:H],
                                            axis=mybir.AxisListType.X, op=mybir.AluOpType.add)
                    nc.vector.tensor_add(out=lsl, in0=pos[:], in1=neg[:])

            for _rep in range(repeat):
                with tc.For_i(0, C, G) as iv:
                    gather_body(iv)

            # ---- sigmoid tail ----
            sig = cpool.tile([P, C], f32)
            nc.scalar.activation(out=sig[:], in_=logits[:],
                                 func=mybir.ActivationFunctionType.Sigmoid,
                                 bias=b2rep_t[:, 0:1], scale=1.0)
            nc.scalar.mul(out=sig[:], in_=sig[:], mul=5.0)
            nc.sync.dma_start(out=out_d[:], in_=sig[:])
    nc.finalize()
    return nc


def _pad_cols(n, mult):
    return ((n + mult - 1) // mult) * mult


def _prepare(z_user, z_movie, edge_index, W1, b1, W2, b2, n_cores=N_CORES):
    z_user = np.asarray(z_user, dtype=np.float32)
    z_movie = np.asarray(z_movie, dtype=np.float32)
    edge_index = np.asarray(edge_index)
    W1 = np.asarray(W1, dtype=np.float32)
    b1 = np.asarray(b1, dtype=np.float32)
    W2 = np.asarray(W2, dtype=np.float32)
    b2 = np.asarray(b2, dtype=np.float32)

    E = edge_index.shape[1]
    rows = edge_index[0].astype(np.int64)
    cols = edge_index[1].astype(np.int64)

    NAr = int(rows.max()) + 1 if E else 1          # referenced user rows
    NBr = z_movie.shape[0]
    NA = _pad_cols(max(NAr, ZBODY), ZBODY)
    NB = _pad_cols(max(NBr, ZBODY), ZBODY)

    # hidden permutation: positive-w2 units first; fold |w2| and b1 into tables
    w2 = W2.reshape(-1)
    perm = np.argsort(w2 < 0, kind="stable")       # stable: positives (False) first
    Hp = int((w2 >= 0).sum())
    W1p = W1[perm]                                  # [H, 2H]
    b1p = b1[perm]
    scale = w2[perm]  # signed: w2*relu(x) = max0(w2*x) for w2>0, min0(w2*x) for w2<0
    w1ut = np.ascontiguousarray((W1p[:, :H] * scale[:, None]).T)   # [in, h]
    w1mt = np.ascontiguousarray((W1p[:, H:] * scale[:, None]).T)
    b1rep = np.tile(b1p * scale, (P, 1)).astype(np.float32)
    b2rep = np.full((P, 1), float(b2.reshape(-1)[0]), dtype=np.float32)

    # transposed, padded node features
    zTu = np.zeros((P, NA), dtype=np.float32)
    zTu[:, :NAr] = z_user[:NAr].T
    zTm = np.zeros((P, NB), dtype=np.float32)
    zTm[:, :NBr] = z_movie.T

    # tile-linearized table row index: u -> (u%128)*(N/128) + u//128
    mA, mB = NA // P, NB // P
    idxA_full = ((rows % P) * mA + rows // P).astype(np.int32)
    idxB_full = ((cols % P) * mB + cols // P).astype(np.int32)

    # shard edges: per core 128*C edges, C divisible by G
    C = _pad_cols(-(-E // (n_cores * P)), G)
    Epc = P * C
    Etot = n_cores * Epc
    idxA_pad = np.zeros(Etot, dtype=np.int32)
    idxA_pad[:E] = idxA_full
    idxB_pad = np.zeros(Etot, dtype=np.int32)
    idxB_pad[:E] = idxB_full

    in_maps = []
    for c in range(n_cores):
        sl = slice(c * Epc, (c + 1) * Epc)
        in_maps.append({
            "zTu": zTu, "zTm": zTm, "w1ut": w1ut, "w1mt": w1mt,
            "b1rep": b1rep, "b2rep": b2rep,
            "idxA": idxA_pad[sl].reshape(P, C),
            "idxB": idxB_pad[sl].reshape(P, C),
        })
    return in_maps, dict(C=C, NA=NA, NB=NB, Hp=Hp, E=E)


def kernel(z_user, z_movie, edge_index, W1, b1, W2, b2):
    in_maps, meta = _prepare(z_user, z_movie, edge_index, W1, b1, W2, b2)
    nc = _build_nc(meta["C"], meta["NA"], meta["NB"], meta["Hp"])
    res = run_bass_kernel_spmd(nc, in_maps, core_ids=list(range(N_CORES)))
    out = np.concatenate([res.results[c]["out"].reshape(-1) for c in range(N_CORES)])
    _LAST_STATS["exec_time_ns"] = res.exec_time_ns
    _LAST_STATS["nc"] = nc
    _LAST_STATS["in_maps"] = in_maps
    _LAST_STATS["meta"] = meta
    return out[:meta["E"]].astype(np.float32)
